# revision 12
# baseline (speedup 1.0000x reference)
"""Trainium2 Bass kernel for dilated 3-tap per-channel softmax attention.

Reference computation (per batch):
    q = wq @ x                      # [O, T]
    xp = pad(x, d=4 both sides)     # [C, T+8]
    k = wk @ xp; v = wv @ xp        # [O, T+8]
    taps at offsets 0, 4, 8 (== t-4, t, t+4 in unpadded coords)
    scores = q * k_tap (per channel), softmax over the 3 taps,
    out = sum(attn * v_tap)         # [O, T]

Sharding: data-parallel over batch, 2 batches per core, 8 cores, no
collectives.  Key algebra: with D(j) = K(j) - K(j+4) over padded coords,
  s0 = q * (k0-k1) =  q * D(t),   s2 = q * (k2-k1) = -q * D(t+4)
so one DVE subtract serves both score taps.  Softmax over {s0, 0, s2}:
  out = (r0*v0 + v1 + r2*v2) / (1 + r0 + r2),  r_i = exp(s_i)
Division via ACT:  1/den = exp(-ln(den)), with the +1 folded into Ln's
bias.  Matmuls run in float32r (TF32-like, full PE rate at N>=256).
"""

import sys
from contextlib import ExitStack

if "/opt/trn_rl_repo" not in sys.path:
    sys.path.insert(0, "/opt/trn_rl_repo")

import numpy as np

import concourse.bacc as bacc
import concourse.tile as tile
import concourse.mybir as mybir
from concourse.bass_utils import run_bass_kernel_spmd

B, C, T = 16, 512, 2048
O = 512
D = 4  # dilation == padding
TP = T + 2 * D  # padded time length, 2056
NCORES = 8
BPC = B // NCORES  # batches per core

KC = C // 128  # contraction chunks
MC = O // 128  # output-channel chunks
TT = 1024  # output cols per tile-iteration
TK = TT + 2 * D  # k/v psum tile cols (1032)

F32 = mybir.dt.float32
F32R = mybir.dt.float32r
BF16 = mybir.dt.bfloat16
FP16 = mybir.dt.float16
AF = mybir.ActivationFunctionType
ALU = mybir.AluOpType

_CACHED = {}


def build_program(reps=1, gp_ops=frozenset()):
    nc = bacc.Bacc("TRN2", target_bir_lowering=False, debug=False)

    xp_d = [
        nc.dram_tensor(f"xp{b}", [C, TP], F32R, kind="ExternalInput").ap()
        for b in range(BPC)
    ]
    w_d = {
        name: nc.dram_tensor(name, [C, O], F32R, kind="ExternalInput").ap()
        for name in ("wqt", "wkt", "wvt")
    }
    out_d = [
        nc.dram_tensor(f"out{b}", [O, T], BF16, kind="ExternalOutput").ap()
        for b in range(BPC)
    ]

    with tile.TileContext(nc) as tc, ExitStack() as ctx:
        wpool = ctx.enter_context(tc.tile_pool(name="w", bufs=1))
        xpool = ctx.enter_context(tc.tile_pool(name="x", bufs=2))
        spool = ctx.enter_context(tc.tile_pool(name="s", bufs=2))
        opool = ctx.enter_context(tc.tile_pool(name="o", bufs=2))
        kpp = ctx.enter_context(tc.tile_pool(name="kp", bufs=1, space="PSUM"))
        qpp = ctx.enter_context(tc.tile_pool(name="qp", bufs=1, space="PSUM"))
        vpp = ctx.enter_context(tc.tile_pool(name="vp", bufs=1, space="PSUM"))

        # weights resident for the whole kernel: [kc][128, O] per projection
        wsb = {}
        for name in ("wqt", "wkt", "wvt"):
            tiles = []
            for kc in range(KC):
                wt = wpool.tile([128, O], F32R, tag=f"{name}{kc}")
                nc.sync.dma_start(wt[:], w_d[name][kc * 128 : (kc + 1) * 128, :])
                tiles.append(wt)
            wsb[name] = tiles

        def _one_pass(_iv=None):
          for b in range(BPC):
            # x for this batch: [kc][128, TP]
              xsb = []
              for kc in range(KC):
                  xt = xpool.tile([128, TP], F32R, tag=f"x{kc}")
                  nc.sync.dma_start(xt[:], xp_d[b][kc * 128 : (kc + 1) * 128, :])
                  xsb.append(xt)

              for m in range(MC):
                  ms = slice(m * 128, (m + 1) * 128)
                  for th in range(0, T, TT):
                      # ---- PE: K over padded cols [th, th+TK) ----
                      kp = kpp.tile([128, TK], F32, tag="kp")
                      for n0, nn in ((0, 512), (512, 512), (1024, TK - 1024)):
                          for kc in range(KC):
                              nc.tensor.matmul(
                                  kp[:, n0 : n0 + nn],
                                  wsb["wkt"][kc][:, ms],
                                  xsb[kc][:, th + n0 : th + n0 + nn],
                                  start=(kc == 0),
                                  stop=(kc == KC - 1),
                              )
                      # evacuate K to SBUF (only one PSUM operand allowed per
                      # DVE op), then D(j) = K(j) - K(j+4)
                      ksb = spool.tile([128, TK], FP16, tag="ksb")
                      nc.scalar.activation(ksb[:], kp[:], AF.Copy)
                      Dt = spool.tile([128, TT + 4], FP16, tag="D")
                      (nc.gpsimd if "D" in gp_ops else nc.vector).tensor_tensor(
                          Dt[:], ksb[:, 0 : TT + 4], ksb[:, 4 : TT + 8], ALU.subtract
                      )

                      # ---- PE: Q over cols [th, th+TT) (padded offset +4) ----
                      qp = qpp.tile([128, TT], F32, tag="qp")
                      for n0 in range(0, TT, 512):
                          for kc in range(KC):
                              nc.tensor.matmul(
                                  qp[:, n0 : n0 + 512],
                                  wsb["wqt"][kc][:, ms],
                                  xsb[kc][:, th + 4 + n0 : th + 4 + n0 + 512],
                                  start=(kc == 0),
                                  stop=(kc == KC - 1),
                              )
                      # scores: s0 = q*D[0:TT], s2n = -q*D[4:TT+4] (one exp later)
                      sb2 = spool.tile([128, 2 * TT], F32, tag="sboth")
                      nc.vector.tensor_tensor(
                          sb2[:, 0:TT], qp[:], Dt[:, 0:TT], ALU.mult
                      )
                      nc.vector.scalar_tensor_tensor(
                          sb2[:, TT : 2 * TT],
                          qp[:],
                          -1.0,
                          Dt[:, 4 : TT + 4],
                          ALU.mult,
                          ALU.mult,
                      )

                      # ---- PE: V over padded cols [th, th+TK) ----
                      vp = vpp.tile([128, TK], F32, tag="vp")
                      for n0, nn in ((0, 512), (512, 512), (1024, TK - 1024)):
                          for kc in range(KC):
                              nc.tensor.matmul(
                                  vp[:, n0 : n0 + nn],
                                  wsb["wvt"][kc][:, ms],
                                  xsb[kc][:, th + n0 : th + n0 + nn],
                                  start=(kc == 0),
                                  stop=(kc == KC - 1),
                              )
                      vb = spool.tile([128, TK], BF16, tag="vb")
                      nc.scalar.activation(vb[:], vp[:], AF.Copy)

                      # exp of both score blocks in one ACT op -> bf16
                      rb = spool.tile([128, 2 * TT], BF16, tag="rboth")
                      nc.scalar.activation(rb[:], sb2[:], AF.Exp)
                      r0 = rb[:, 0:TT]
                      r2 = rb[:, TT : 2 * TT]

                      # denominator: den = r0 + r2; 1/(1+den) via Ln/Exp
                      den = spool.tile([128, TT], BF16, tag="den")
                      (nc.gpsimd if "den" in gp_ops else nc.vector).tensor_tensor(den[:], r0, r2, ALU.add)
                      Lt = spool.tile([128, TT], F32, tag="L")
                      nc.scalar.activation(Lt[:], den[:], AF.Ln, bias=1.0)
                      inv = spool.tile([128, TT], BF16, tag="inv")
                      nc.scalar.activation(inv[:], Lt[:], AF.Exp, scale=-1.0)

                      # numerator: r0*v0 + v1 + r2*v2
                      t0 = spool.tile([128, TT], BF16, tag="t0")
                      (nc.gpsimd if "t0" in gp_ops else nc.vector).tensor_tensor(t0[:], r0, vb[:, 0:TT], ALU.mult)
                      t2 = spool.tile([128, TT], BF16, tag="t2")
                      (nc.gpsimd if "t2" in gp_ops else nc.vector).tensor_tensor(t2[:], r2, vb[:, 8 : TT + 8], ALU.mult)
                      nt = spool.tile([128, TT], BF16, tag="nt")
                      (nc.gpsimd if "nt" in gp_ops else nc.vector).tensor_tensor(nt[:], t0[:], t2[:], ALU.add)
                      num = spool.tile([128, TT], BF16, tag="num")
                      (nc.gpsimd if "num" in gp_ops else nc.vector).tensor_tensor(num[:], nt[:], vb[:, 4 : TT + 4], ALU.add)

                      # out = num * inv (bf16; host upconverts to fp32)
                      ot = opool.tile([128, TT], BF16, tag="out")
                      (nc.gpsimd if "final" in gp_ops else nc.vector).tensor_tensor(ot[:], num[:], inv[:], ALU.mult)
                      nc.sync.dma_start(out_d[b][ms, th : th + TT], ot[:])

        if reps == 1:
            _one_pass()
        else:
            with tc.For_i(0, reps, 1) as iv:
                _one_pass(iv)

    # All ACT funcs used (Copy/Exp/Ln) live in one table set; restricting
    # the candidate list stops the table-load inserter from alternating
    # between exp_and_others and natural_log every iteration (~2.7us/load).
    import concourse.bacc as _bacc_mod
    _orig_get_tables = _bacc_mod.get_activation_tables

    def _only_ln_exp(arch):
        # act_func_set_id is the positional index into this dict, so keep
        # every entry in place and just blank the sets we don't want picked.
        t = _orig_get_tables(arch)
        return {
            name: (fns if name == "natural_log_exp_and_others" else set())
            for name, fns in t.items()
        }

    _bacc_mod.get_activation_tables = _only_ln_exp
    try:
        nc.finalize()
    finally:
        _bacc_mod.get_activation_tables = _orig_get_tables
    return nc


def kernel(x, wq, wk, wv):
    x = np.asarray(x)
    wq, wk, wv = np.asarray(wq), np.asarray(wk), np.asarray(wv)
    assert x.shape == (B, C, T) and x.dtype == np.float32
    if "nc" not in _CACHED:
        _CACHED["nc"] = build_program()
    nc = _CACHED["nc"]

    xpad = np.zeros((B, C, TP), dtype=np.float32)
    xpad[:, :, D : D + T] = x
    wqt = np.ascontiguousarray(wq.T)  # [C, O]
    wkt = np.ascontiguousarray(wk.T)
    wvt = np.ascontiguousarray(wv.T)

    in_maps = []
    for core in range(NCORES):
        m = {"wqt": wqt, "wkt": wkt, "wvt": wvt}
        for b in range(BPC):
            m[f"xp{b}"] = np.ascontiguousarray(xpad[core * BPC + b])
        in_maps.append(m)

    trace = bool(_CACHED.get("trace"))
    res = run_bass_kernel_spmd(
        nc, in_maps, core_ids=list(range(NCORES)), trace=trace
    )
    if trace:
        _CACHED["last_exec_time_ns"] = res.exec_time_ns
        _CACHED["last_results"] = res
    out = np.empty((B, O, T), dtype=np.float32)
    for core in range(NCORES):
        for b in range(BPC):
            out[core * BPC + b] = res.results[core][f"out{b}"].astype(np.float32)
    return out

